# revision 5
# baseline (speedup 1.0000x reference)
"""Trainium2 Bass kernel for nn_CausalFieldAttention.

Shapes (hardcoded): B=4, N=4096, D=1024, H=16, hd=64, G=512, sigma=3.

Reference computation (the q-projection is computed but unused -> skipped):
    k  = x @ k_w.T + k_b                      (B,N,D) -> heads (B,H,N,hd)
    v  = x @ v_w.T + v_b
    wv = v * ||k||_head                       per-token, per-head scale
    field = segment_sum(wv, field_idx, G)     scatter tokens -> G bins
    conv  = circular_conv(field, causal_ker)  (reference: via rfft/irfft)
    y  = conv[field_idx]                      gather bins -> tokens
    out = y @ out_w.T + out_b

Device strategy: 8 cores = 4 batches x 2 head-groups (8 heads / 512 channels
each). Per core, everything is PE matmuls in f32r (full-rate fp32 mode):
  - k/v projections: (tok x ch) psum tiles, contraction over D.
  - ||k||: ACT square+accum per head, ACT sqrt; wv: DVE tensor_scalar.
  - scatter: block-sparse 0/1 matrix S (tokens are sorted by bin, so each
    128-token tile touches ~17 consecutive bins => ~1 matmul per tile).
  - circular conv: exact circulant matmul (replaces the FFT; only the
    gt->gt+1, gt+2 128x128 blocks are numerically nonzero).
  - gather fused with transpose: yT = conv.T @ S, giving y in (ch x tok)
    layout, which feeds the out-projection directly.
  - out-projection: per-core partial over its 512 channels, output (D x N).
Host: out[b] = partialT[2b].T + partialT[2b+1].T + out_b.
"""

import os
import sys
from contextlib import ExitStack

import numpy as np

for _p in ("/opt/trn_rl_repo", "/root/.axon_site/_ro/trn_rl_repo"):
    if os.path.isdir(_p) and _p not in sys.path:
        sys.path.append(_p)

import concourse.bacc as bacc
import concourse.mybir as mybir
import concourse.tile as tile
from concourse.bass_utils import run_bass_kernel_spmd

B, N, D = 4, 4096, 1024
H, HD, G = 16, 64, 512
SIGMA = 3.0
P = 128
KT = D // P          # 8 contraction tiles over D
TT = N // P          # 32 token tiles
GT = G // P          # 4 bin tiles
NCH = N // 512       # 8 token chunks of 512
CLOC = 512           # channels per core (8 heads)
HLOC = CLOC // HD    # 8 heads per core
NCORES = 8

F32 = mybir.dt.float32
F32R = mybir.dt.float32r

# set by test harness to capture a profile; kernel() stores results here
TRACE = False
LAST_RESULT = None


def _field_idx():
    # exactly mirrors the reference (fp32 div then mul, trunc, clip)
    pos = np.arange(N, dtype=np.float32) / np.float32(N - 1) * np.float32(G - 1)
    return np.clip(pos.astype(np.int32), 0, G - 1)


def _causal_kernel():
    i = np.arange(G)
    dist = np.abs(i - G // 2)
    ker = np.where(i >= G // 2, 0.0, np.exp(-dist / SIGMA)).astype(np.float32)
    ker = ker / (ker.sum() + 1e-8)
    return ker


def _plans():
    idx = _field_idx()
    ker = _causal_kernel()
    gg = (np.arange(G)[None, :] - np.arange(G)[:, None]) % G  # CT[g, g2] = ker[(g2-g)%G]
    CTm = ker[gg].astype(np.float32)

    Smat = np.zeros((N, G), np.float32)
    Smat[np.arange(N), idx] = 1.0
    STm = np.ascontiguousarray(Smat.T)

    tt_gts = [sorted(set((idx[t * P:(t + 1) * P] // P).tolist())) for t in range(TT)]
    contribs = {gt: [t for t in range(TT) if gt in tt_gts[t]] for gt in range(GT)}
    scatter_plan = [
        [(gt, t == contribs[gt][0], t == contribs[gt][-1]) for gt in tt_gts[t]]
        for t in range(TT)
    ]
    gather_plan = [
        sorted(set((idx[c * 512:(c + 1) * 512] // P).tolist())) for c in range(NCH)
    ]
    conv_blocks = [
        [gt for gt in range(GT)
         if np.abs(CTm[gt * P:(gt + 1) * P, gp * P:(gp + 1) * P]).max() > 1e-12]
        for gp in range(GT)
    ]
    return idx, CTm, Smat, STm, scatter_plan, gather_plan, conv_blocks


def _build_program(with_kb, with_vb, scatter_plan, gather_plan, conv_blocks):
    nc = bacc.Bacc("TRN2", target_bir_lowering=False, debug=False,
                   num_devices=NCORES)
    xT = nc.dram_tensor("xT", [D, N], F32R, kind="ExternalInput").ap()
    kwT = nc.dram_tensor("kwT", [D, CLOC], F32R, kind="ExternalInput").ap()
    vwT = nc.dram_tensor("vwT", [D, CLOC], F32R, kind="ExternalInput").ap()
    owT = nc.dram_tensor("owT", [CLOC, D], F32R, kind="ExternalInput").ap()
    Sm = nc.dram_tensor("Smat", [N, G], F32R, kind="ExternalInput").ap()
    STm = nc.dram_tensor("STm", [G, N], F32R, kind="ExternalInput").ap()
    CTm = nc.dram_tensor("CTm", [G, G], F32R, kind="ExternalInput").ap()
    kb = nc.dram_tensor("kb", [1, CLOC], F32R, kind="ExternalInput").ap() if with_kb else None
    vb = nc.dram_tensor("vb", [1, CLOC], F32R, kind="ExternalInput").ap() if with_vb else None
    outT = nc.dram_tensor("outT", [D, N], F32, kind="ExternalOutput").ap()

    with tile.TileContext(nc) as tc, ExitStack() as es:
        cpool = es.enter_context(tc.tile_pool(name="const", bufs=1))

        kw_sb = cpool.tile([P, KT, CLOC], F32R)
        nc.sync.dma_start(kw_sb[:], kwT.rearrange("(kt p) c -> p kt c", p=P))
        vw_sb = cpool.tile([P, KT, CLOC], F32R)
        nc.sync.dma_start(vw_sb[:], vwT.rearrange("(kt p) c -> p kt c", p=P))
        ow_sb = cpool.tile([P, GT, D], F32R)
        nc.sync.dma_start(ow_sb[:], owT.rearrange("(ct p) e -> p ct e", p=P))
        ct_sb = cpool.tile([P, GT, G], F32R)
        nc.sync.dma_start(ct_sb[:], CTm.rearrange("(gt p) g2 -> p gt g2", p=P))
        field_sb = cpool.tile([P, GT, G], F32R)
        conv_sb = cpool.tile([P, GT, G], F32R)
        if with_kb or with_vb:
            ones_sb = cpool.tile([1, P], F32R)
            nc.vector.memset(ones_sb[:], 1.0)
        if with_kb:
            kb_sb = cpool.tile([1, CLOC], F32R)
            nc.sync.dma_start(kb_sb[:], kb[:])
        if with_vb:
            vb_sb = cpool.tile([1, CLOC], F32R)
            nc.sync.dma_start(vb_sb[:], vb[:])

        # ---- phase 1: projections, wv, scatter ----
        ph1 = ExitStack()
        xpool = ph1.enter_context(tc.tile_pool(name="xin", bufs=3))
        spool = ph1.enter_context(tc.tile_pool(name="sblk", bufs=3))
        wvpool = ph1.enter_context(tc.tile_pool(name="wv", bufs=3))
        smpool = ph1.enter_context(tc.tile_pool(name="small", bufs=3))
        ps_k = ph1.enter_context(tc.tile_pool(name="ps_k", bufs=2, space="PSUM"))
        ps_v = ph1.enter_context(tc.tile_pool(name="ps_v", bufs=2, space="PSUM"))
        ps_f = ph1.enter_context(tc.tile_pool(name="ps_f", bufs=1, space="PSUM"))

        field_ps = [ps_f.tile([P, CLOC], F32, tag=f"fld{gt}", name=f"fld{gt}")
                    for gt in range(GT)]
        xT_r = xT.rearrange("(kt p) n -> p kt n", p=P)

        def emit_scatter(tt, wv):
            tsl = slice(tt * P, (tt + 1) * P)
            for gt, first, last in scatter_plan[tt]:
                sblk = spool.tile([P, P], F32R, tag="sblk")
                nc.sync.dma_start(sblk[:], Sm[tsl, gt * P:(gt + 1) * P])
                nc.tensor.matmul(field_ps[gt][:], sblk[:], wv[:],
                                 start=first, stop=last)

        pending = None
        for tt in range(TT):
            tsl = slice(tt * P, (tt + 1) * P)
            xb = xpool.tile([P, KT, P], F32R, tag="xblk")
            nc.sync.dma_start(xb[:], xT_r[:, :, tsl])

            kps = ps_k.tile([P, CLOC], F32, tag="kps")
            vps = ps_v.tile([P, CLOC], F32, tag="vps")
            for kt in range(KT):
                lhs = xb[:, kt, :]
                nc.tensor.matmul(kps[:], lhs, kw_sb[:, kt, :],
                                 start=(kt == 0), stop=(kt == KT - 1 and not with_kb))
                nc.tensor.matmul(vps[:], lhs, vw_sb[:, kt, :],
                                 start=(kt == 0), stop=(kt == KT - 1 and not with_vb))
            if with_kb:
                nc.tensor.matmul(kps[:], ones_sb[:], kb_sb[:],
                                 start=False, stop=True)
            if with_vb:
                nc.tensor.matmul(vps[:], ones_sb[:], vb_sb[:],
                                 start=False, stop=True)

            # scatter for the previous tile (keeps PE dense: its wv is ready)
            if pending is not None:
                emit_scatter(*pending)

            # ||k|| per head: ACT square w/ accumulate, then sqrt
            km2 = smpool.tile([P, HLOC], F32, tag="km2")
            for h in range(HLOC):
                hs = slice(h * HD, (h + 1) * HD)
                scr = smpool.tile([P, HD], F32, tag="sqscr")
                nc.scalar.activation(scr[:], kps[:, hs],
                                     mybir.ActivationFunctionType.Square,
                                     accum_out=km2[:, h:h + 1])
            km = smpool.tile([P, HLOC], F32, tag="km")
            nc.scalar.sqrt(km[:], km2[:])

            wv = wvpool.tile([P, CLOC], F32R, tag="wv")
            for h in range(HLOC):
                hs = slice(h * HD, (h + 1) * HD)
                nc.vector.tensor_scalar_mul(wv[:, hs], vps[:, hs], km[:, h:h + 1])
            pending = (tt, wv)

        emit_scatter(*pending)
        for gt in range(GT):
            nc.vector.tensor_copy(field_sb[:, gt, :], field_ps[gt][:])
        ph1.close()

        # ---- phase 2: circular conv (circulant matmul) ----
        ph2 = ExitStack()
        ps_c = ph2.enter_context(tc.tile_pool(name="ps_c", bufs=2, space="PSUM"))
        for gp in range(GT):
            cps = ps_c.tile([P, CLOC], F32, tag="cps")
            blocks = conv_blocks[gp]
            for i, gt in enumerate(blocks):
                nc.tensor.matmul(cps[:], ct_sb[:, gt, gp * P:(gp + 1) * P],
                                 field_sb[:, gt, :],
                                 start=(i == 0), stop=(i == len(blocks) - 1))
            nc.vector.tensor_copy(conv_sb[:, gp, :], cps[:])
        ph2.close()

        # ---- phase 3: gather (transposed) + out-projection ----
        ph3 = ExitStack()
        stpool = ph3.enter_context(tc.tile_pool(name="st_in", bufs=3))
        ypool = ph3.enter_context(tc.tile_pool(name="ych", bufs=2))
        opool = ph3.enter_context(tc.tile_pool(name="osb", bufs=4))
        ps_y = ph3.enter_context(tc.tile_pool(name="ps_y", bufs=4, space="PSUM"))
        ps_o = ph3.enter_context(tc.tile_pool(name="ps_o", bufs=3, space="PSUM"))

        for tci in range(NCH):
            tsl = slice(tci * 512, (tci + 1) * 512)
            gts = gather_plan[tci]
            st_tiles = {}
            for gt in gts:
                st = stpool.tile([P, 512], F32R, tag="st")
                nc.sync.dma_start(st[:], STm[gt * P:(gt + 1) * P, tsl])
                st_tiles[gt] = st
            ych = ypool.tile([P, GT, 512], F32R, tag="ych")
            for ct in range(GT):
                yps = ps_y.tile([P, 512], F32, tag="yps")
                for i, gt in enumerate(gts):
                    nc.tensor.matmul(yps[:], conv_sb[:, gt, ct * P:(ct + 1) * P],
                                     st_tiles[gt][:],
                                     start=(i == 0), stop=(i == len(gts) - 1))
                nc.vector.tensor_copy(ych[:, ct, :], yps[:])
            for et in range(KT):
                ops = ps_o.tile([P, 512], F32, tag="ops")
                for ct in range(GT):
                    nc.tensor.matmul(ops[:], ow_sb[:, ct, et * P:(et + 1) * P],
                                     ych[:, ct, :],
                                     start=(ct == 0), stop=(ct == GT - 1))
                osb = opool.tile([P, 512], F32, tag="osb")
                nc.vector.tensor_copy(osb[:], ops[:])
                nc.sync.dma_start(outT[et * P:(et + 1) * P, tsl], osb[:])
        ph3.close()

    nc.compile()
    return nc


_PROGRAM_CACHE = {}


def _get_program(with_kb, with_vb):
    key = (with_kb, with_vb)
    if key not in _PROGRAM_CACHE:
        _, _, _, _, sp, gp, cb = _plans()
        _PROGRAM_CACHE[key] = _build_program(with_kb, with_vb, sp, gp, cb)
    return _PROGRAM_CACHE[key]


def kernel(x, q_w, q_b, k_w, k_b, v_w, v_b, out_w, out_b):
    global LAST_RESULT
    x = np.asarray(x, dtype=np.float32)
    k_w = np.asarray(k_w, dtype=np.float32)
    k_b = np.asarray(k_b, dtype=np.float32)
    v_w = np.asarray(v_w, dtype=np.float32)
    v_b = np.asarray(v_b, dtype=np.float32)
    out_w = np.asarray(out_w, dtype=np.float32)
    out_b = np.asarray(out_b, dtype=np.float32)

    with_kb = bool(np.any(k_b))
    with_vb = bool(np.any(v_b))
    nc = _get_program(with_kb, with_vb)
    _, CTm, Smat, STm, _, _, _ = _plans()

    in_maps = []
    for c in range(NCORES):
        b, hg = c // 2, c % 2
        chs = slice(hg * CLOC, (hg + 1) * CLOC)
        m = {
            "xT": np.ascontiguousarray(x[b].T),
            "kwT": np.ascontiguousarray(k_w[chs, :].T),
            "vwT": np.ascontiguousarray(v_w[chs, :].T),
            "owT": np.ascontiguousarray(out_w[:, chs].T),
            "Smat": Smat,
            "STm": STm,
            "CTm": CTm,
        }
        if with_kb:
            m["kb"] = np.ascontiguousarray(k_b[chs][None, :])
        if with_vb:
            m["vb"] = np.ascontiguousarray(v_b[chs][None, :])
        in_maps.append(m)

    res = run_bass_kernel_spmd(nc, in_maps, core_ids=list(range(NCORES)),
                               trace=TRACE)
    LAST_RESULT = res

    out = np.empty((B, N, D), dtype=np.float32)
    for b in range(B):
        acc = res.results[2 * b]["outT"] + res.results[2 * b + 1]["outT"]
        out[b] = acc.T + out_b[None, :]
    return out


# revision 7
# speedup vs baseline: 1.0941x; 1.0941x over previous
"""Trainium2 Bass kernel for nn_CausalFieldAttention.

Shapes (hardcoded): B=4, N=4096, D=1024, H=16, hd=64, G=512, sigma=3.

Reference computation (the q-projection is computed but unused -> skipped):
    k  = x @ k_w.T + k_b                      (B,N,D) -> heads (B,H,N,hd)
    v  = x @ v_w.T + v_b
    wv = v * ||k||_head                       per-token, per-head scale
    field = segment_sum(wv, field_idx, G)     scatter tokens -> G bins
    conv  = circular_conv(field, causal_ker)  (reference: via rfft/irfft)
    y  = conv[field_idx]                      gather bins -> tokens
    out = y @ out_w.T + out_b

Device strategy: 8 cores = 4 batches x 2 head-groups (8 heads / 512 channels
each), everything in f32r (full-rate fp32 matmul mode):
  - k/v projections: (tok x ch) psum tiles, contraction over D.
  - ||k||: one ACT Square per token tile + DVE grouped reduce + ACT sqrt;
    wv = v * ||k|| as one DVE multiply with a stride-0 broadcast AP.
  - scatter: block-sparse 0/1 matrix S; tokens are sorted by bin, so each
    128-token tile hits ~17 consecutive bins => ~1 matmul per tile.
  - circular conv: exact circulant matmul, produced transposed:
    convT = field.T @ C.T (the FFT in the reference is just this, exactly).
  - KEY reassociation: out = gather(conv) @ out_w = gather(conv @ out_w).
    A = conv @ ow is computed once at bin granularity (512 rows instead of
    4096), then the gather IS the final matmul: out(t,e) = S.T @ A.
  - out-projection partial per core over its 512 channels; host sums the
    two head-group partials per batch and adds out_b.
"""

import os
import sys
from contextlib import ExitStack

import numpy as np

for _p in ("/opt/trn_rl_repo", "/root/.axon_site/_ro/trn_rl_repo"):
    if os.path.isdir(_p) and _p not in sys.path:
        sys.path.append(_p)

import concourse.bacc as bacc
import concourse.mybir as mybir
import concourse.tile as tile
from concourse.bass_utils import run_bass_kernel_spmd

B, N, D = 4, 4096, 1024
H, HD, G = 16, 64, 512
SIGMA = 3.0
P = 128
KT = D // P          # 8 contraction tiles over D
TT = N // P          # 32 token tiles
GT = G // P          # 4 bin tiles
CLOC = 512           # channels per core (8 heads)
HLOC = CLOC // HD    # 8 heads per core
ECH = D // 512       # 2 chunks of out-channels for 512-wide psum
NCORES = 8

F32 = mybir.dt.float32
F32R = mybir.dt.float32r

# set by test harness to capture a profile; kernel() stores results here
TRACE = False
LAST_RESULT = None


def _field_idx():
    # exactly mirrors the reference (fp32 div then mul, trunc, clip)
    pos = np.arange(N, dtype=np.float32) / np.float32(N - 1) * np.float32(G - 1)
    return np.clip(pos.astype(np.int32), 0, G - 1)


def _causal_kernel():
    i = np.arange(G)
    dist = np.abs(i - G // 2)
    ker = np.where(i >= G // 2, 0.0, np.exp(-dist / SIGMA)).astype(np.float32)
    ker = ker / (ker.sum() + 1e-8)
    return ker


def _plans():
    idx = _field_idx()
    ker = _causal_kernel()
    gg = (np.arange(G)[None, :] - np.arange(G)[:, None]) % G  # CT[g, g2] = ker[(g2-g)%G]
    CTm = ker[gg].astype(np.float32)

    Smat = np.zeros((N, G), np.float32)
    Smat[np.arange(N), idx] = 1.0
    STm = np.ascontiguousarray(Smat.T)

    tt_gts = [sorted(set((idx[t * P:(t + 1) * P] // P).tolist())) for t in range(TT)]
    contribs = {gt: [t for t in range(TT) if gt in tt_gts[t]] for gt in range(GT)}
    scatter_plan = [
        [(gt, t == contribs[gt][0], t == contribs[gt][-1]) for gt in tt_gts[t]]
        for t in range(TT)
    ]
    return idx, CTm, Smat, STm, scatter_plan, tt_gts


def _build_program(with_kb, with_vb, scatter_plan, tt_gts):
    nc = bacc.Bacc("TRN2", target_bir_lowering=False, debug=False,
                   num_devices=NCORES)
    xT = nc.dram_tensor("xT", [D, N], F32R, kind="ExternalInput").ap()
    kwT = nc.dram_tensor("kwT", [D, CLOC], F32R, kind="ExternalInput").ap()
    vwT = nc.dram_tensor("vwT", [D, CLOC], F32R, kind="ExternalInput").ap()
    owT = nc.dram_tensor("owT", [CLOC, D], F32R, kind="ExternalInput").ap()
    Sm = nc.dram_tensor("Smat", [N, G], F32R, kind="ExternalInput").ap()
    STmat = nc.dram_tensor("STm", [G, N], F32R, kind="ExternalInput").ap()
    CTmat = nc.dram_tensor("CTm", [G, G], F32R, kind="ExternalInput").ap()
    kb = nc.dram_tensor("kb", [1, CLOC], F32R, kind="ExternalInput").ap() if with_kb else None
    vb = nc.dram_tensor("vb", [1, CLOC], F32R, kind="ExternalInput").ap() if with_vb else None
    out_d = nc.dram_tensor("out", [N, D], F32, kind="ExternalOutput").ap()

    xT_r = xT.rearrange("(kt p) n -> p kt n", p=P)
    kwT_r = kwT.rearrange("(kt p) c -> p kt c", p=P)
    vwT_r = vwT.rearrange("(kt p) c -> p kt c", p=P)

    with tile.TileContext(nc) as tc, ExitStack() as es:
        cpool = es.enter_context(tc.tile_pool(name="const", bufs=1))

        # resident tensors; k/v weights split per-kt so the first projection
        # matmuls only wait on their own 256KB slice (subtile deps)
        kw_sb = cpool.tile([P, KT, CLOC], F32R)
        vw_sb = cpool.tile([P, KT, CLOC], F32R)
        nc.sync.dma_start(kw_sb[:, 0, :], kwT_r[:, 0, :])
        nc.scalar.dma_start(vw_sb[:, 0, :], vwT_r[:, 0, :])
        for kt in range(1, KT):
            nc.sync.dma_start(kw_sb[:, kt, :], kwT_r[:, kt, :])
            nc.scalar.dma_start(vw_sb[:, kt, :], vwT_r[:, kt, :])
        ow_sb = cpool.tile([P, GT, D], F32R)
        nc.gpsimd.dma_start(ow_sb[:], owT.rearrange("(ct p) e -> p ct e", p=P))
        ct_sb = cpool.tile([P, GT, G], F32R)
        nc.gpsimd.dma_start(ct_sb[:], CTmat.rearrange("(gt p) g2 -> p gt g2", p=P))
        field_sb = cpool.tile([P, GT, G], F32R)
        convT_sb = cpool.tile([P, GT, G], F32R)
        A_sb = cpool.tile([P, GT, D], F32R)
        if with_kb or with_vb:
            ones_sb = cpool.tile([1, P], F32R)
            nc.vector.memset(ones_sb[:], 1.0)
        if with_kb:
            kb_sb = cpool.tile([1, CLOC], F32R)
            nc.sync.dma_start(kb_sb[:], kb[:])
        if with_vb:
            vb_sb = cpool.tile([1, CLOC], F32R)
            nc.sync.dma_start(vb_sb[:], vb[:])

        # ---- phase 1: projections, ||k||, wv, scatter ----
        ph1 = ExitStack()
        xpool = ph1.enter_context(tc.tile_pool(name="xin", bufs=3))
        spool = ph1.enter_context(tc.tile_pool(name="sblk", bufs=3))
        wvpool = ph1.enter_context(tc.tile_pool(name="wv", bufs=3))
        smpool = ph1.enter_context(tc.tile_pool(name="small", bufs=3))
        ps_k = ph1.enter_context(tc.tile_pool(name="ps_k", bufs=2, space="PSUM"))
        ps_v = ph1.enter_context(tc.tile_pool(name="ps_v", bufs=2, space="PSUM"))
        ps_f = ph1.enter_context(tc.tile_pool(name="ps_f", bufs=1, space="PSUM"))

        field_ps = [ps_f.tile([P, CLOC], F32, tag=f"fld{gt}", name=f"fld{gt}")
                    for gt in range(GT)]

        def emit_scatter(tt, wv):
            tsl = slice(tt * P, (tt + 1) * P)
            for gt, first, last in scatter_plan[tt]:
                sblk = spool.tile([P, P], F32R, tag="sblk")
                nc.gpsimd.dma_start(sblk[:], Sm[tsl, gt * P:(gt + 1) * P])
                nc.tensor.matmul(field_ps[gt][:], sblk[:], wv[:],
                                 start=first, stop=last)

        pending = None
        for tt in range(TT):
            tsl = slice(tt * P, (tt + 1) * P)
            xb = xpool.tile([P, KT, P], F32R, tag="xblk")
            if tt == 0:
                # split the first load so matmul kt can start after 64KB
                for kt in range(KT):
                    eng = (nc.sync, nc.scalar, nc.gpsimd)[kt % 3]
                    eng.dma_start(xb[:, kt, :], xT_r[:, kt, tsl])
            else:
                nc.sync.dma_start(xb[:], xT_r[:, :, tsl])

            kps = ps_k.tile([P, CLOC], F32, tag="kps")
            vps = ps_v.tile([P, CLOC], F32, tag="vps")
            for kt in range(KT):
                lhs = xb[:, kt, :]
                nc.tensor.matmul(kps[:], lhs, kw_sb[:, kt, :],
                                 start=(kt == 0), stop=(kt == KT - 1 and not with_kb))
                nc.tensor.matmul(vps[:], lhs, vw_sb[:, kt, :],
                                 start=(kt == 0), stop=(kt == KT - 1 and not with_vb))
            if with_kb:
                nc.tensor.matmul(kps[:], ones_sb[:], kb_sb[:], start=False, stop=True)
            if with_vb:
                nc.tensor.matmul(vps[:], ones_sb[:], vb_sb[:], start=False, stop=True)

            # scatter for the previous tile (keeps PE dense: its wv is ready)
            if pending is not None:
                emit_scatter(*pending)

            # ||k|| per head
            ksq = smpool.tile([P, CLOC], F32, tag="ksq")
            nc.scalar.activation(ksq[:], kps[:], mybir.ActivationFunctionType.Square)
            km2 = smpool.tile([P, HLOC], F32, tag="km2")
            nc.vector.reduce_sum(km2[:], ksq[:].rearrange("p (h d) -> p h d", d=HD),
                                 axis=mybir.AxisListType.X)
            km = smpool.tile([P, HLOC], F32, tag="km")
            nc.scalar.sqrt(km[:], km2[:])

            # wv = v * ||k||, one DVE op via stride-0 broadcast of km
            wv = wvpool.tile([P, CLOC], F32R, tag="wv")
            nc.vector.tensor_tensor(
                wv[:].rearrange("p (h d) -> p h d", d=HD),
                vps[:].rearrange("p (h d) -> p h d", d=HD),
                km[:].unsqueeze(2).broadcast_to((P, HLOC, HD)),
                mybir.AluOpType.mult)
            pending = (tt, wv)

        emit_scatter(*pending)
        for gt in range(GT):
            nc.vector.tensor_copy(field_sb[:, gt, :], field_ps[gt][:])
        ph1.close()

        # ---- phase 2: convT = field.T @ C.T, then A = conv @ ow ----
        ph2 = ExitStack()
        ps_2 = ph2.enter_context(tc.tile_pool(name="ps_2", bufs=4, space="PSUM"))
        for ct in range(GT):
            cps = ps_2.tile([P, G], F32, tag="cps")
            for gt in range(GT):
                nc.tensor.matmul(cps[:], field_sb[:, gt, ct * P:(ct + 1) * P],
                                 ct_sb[:, gt, :],
                                 start=(gt == 0), stop=(gt == GT - 1))
            nc.vector.tensor_copy(convT_sb[:, ct, :], cps[:])
        for gp in range(GT):
            for ec in range(ECH):
                aps = ps_2.tile([P, 512], F32, tag="aps")
                esl = slice(ec * 512, (ec + 1) * 512)
                for ct in range(GT):
                    nc.tensor.matmul(aps[:], convT_sb[:, ct, gp * P:(gp + 1) * P],
                                     ow_sb[:, ct, esl],
                                     start=(ct == 0), stop=(ct == GT - 1))
                nc.vector.tensor_copy(A_sb[:, gp, esl], aps[:])
        ph2.close()

        # ---- phase 3: out(t, e) = S.T @ A  (gather == final matmul) ----
        ph3 = ExitStack()
        stpool = ph3.enter_context(tc.tile_pool(name="st_in", bufs=4))
        opool = ph3.enter_context(tc.tile_pool(name="osb", bufs=3))
        ps_o = ph3.enter_context(tc.tile_pool(name="ps_o", bufs=4, space="PSUM"))

        for tt in range(TT):
            tsl = slice(tt * P, (tt + 1) * P)
            gts = tt_gts[tt]
            st_tiles = {}
            for gt in gts:
                st = stpool.tile([P, P], F32R, tag="st")
                nc.gpsimd.dma_start(st[:], STmat[gt * P:(gt + 1) * P, tsl])
                st_tiles[gt] = st
            osb = opool.tile([P, ECH, 512], F32, tag="osb")
            for ec in range(ECH):
                ops = ps_o.tile([P, 512], F32, tag="ops")
                esl = slice(ec * 512, (ec + 1) * 512)
                for i, gt in enumerate(gts):
                    nc.tensor.matmul(ops[:], st_tiles[gt][:], A_sb[:, gt, esl],
                                     start=(i == 0), stop=(i == len(gts) - 1))
                if (tt + ec) % 3 == 0:
                    nc.scalar.copy(osb[:, ec, :], ops[:])
                else:
                    nc.vector.tensor_copy(osb[:, ec, :], ops[:])
            nc.sync.dma_start(out_d[tsl, :], osb[:].rearrange("p ec f -> p (ec f)"))
        ph3.close()

    nc.compile()
    return nc


_PROGRAM_CACHE = {}


def _get_program(with_kb, with_vb):
    key = (with_kb, with_vb)
    if key not in _PROGRAM_CACHE:
        _, _, _, _, sp, tg = _plans()
        _PROGRAM_CACHE[key] = _build_program(with_kb, with_vb, sp, tg)
    return _PROGRAM_CACHE[key]


def kernel(x, q_w, q_b, k_w, k_b, v_w, v_b, out_w, out_b):
    global LAST_RESULT
    x = np.asarray(x, dtype=np.float32)
    k_w = np.asarray(k_w, dtype=np.float32)
    k_b = np.asarray(k_b, dtype=np.float32)
    v_w = np.asarray(v_w, dtype=np.float32)
    v_b = np.asarray(v_b, dtype=np.float32)
    out_w = np.asarray(out_w, dtype=np.float32)
    out_b = np.asarray(out_b, dtype=np.float32)

    with_kb = bool(np.any(k_b))
    with_vb = bool(np.any(v_b))
    nc = _get_program(with_kb, with_vb)
    _, CTm, Smat, STm, _, _ = _plans()

    in_maps = []
    for c in range(NCORES):
        b, hg = c // 2, c % 2
        chs = slice(hg * CLOC, (hg + 1) * CLOC)
        m = {
            "xT": np.ascontiguousarray(x[b].T),
            "kwT": np.ascontiguousarray(k_w[chs, :].T),
            "vwT": np.ascontiguousarray(v_w[chs, :].T),
            "owT": np.ascontiguousarray(out_w[:, chs].T),
            "Smat": Smat,
            "STm": STm,
            "CTm": CTm,
        }
        if with_kb:
            m["kb"] = np.ascontiguousarray(k_b[chs][None, :])
        if with_vb:
            m["vb"] = np.ascontiguousarray(v_b[chs][None, :])
        in_maps.append(m)

    res = run_bass_kernel_spmd(nc, in_maps, core_ids=list(range(NCORES)),
                               trace=TRACE)
    LAST_RESULT = res

    out = np.empty((B, N, D), dtype=np.float32)
    for b in range(B):
        out[b] = res.results[2 * b]["out"] + res.results[2 * b + 1]["out"]
        out[b] += out_b[None, :]
    return out


# revision 9
# speedup vs baseline: 1.1542x; 1.0549x over previous
"""Trainium2 Bass kernel for nn_CausalFieldAttention.

Shapes (hardcoded): B=4, N=4096, D=1024, H=16, hd=64, G=512, sigma=3.

Reference computation (the q-projection is computed but unused -> skipped):
    k  = x @ k_w.T + k_b                      (B,N,D) -> heads (B,H,N,hd)
    v  = x @ v_w.T + v_b
    wv = v * ||k||_head                       per-token, per-head scale
    field = segment_sum(wv, field_idx, G)     scatter tokens -> G bins
    conv  = circular_conv(field, causal_ker)  (reference: via rfft/irfft)
    y  = conv[field_idx]                      gather bins -> tokens
    out = y @ out_w.T + out_b

Device strategy: 8 cores = 4 batches x 2 head-groups (8 heads / 512 channels
each), everything in f32r (full-rate fp32 matmul mode):
  - k/v projections: (tok x ch) psum tiles, contraction over D.
  - ||k||: one ACT Square per token tile + DVE grouped reduce + ACT sqrt;
    wv = v * ||k|| as one DVE multiply with a stride-0 broadcast AP.
  - scatter: block-sparse 0/1 matrix S; tokens are sorted by bin, so each
    128-token tile hits ~17 consecutive bins => ~1 matmul per tile.
  - circular conv: exact circulant matmul, produced transposed:
    convT = field.T @ C.T (the FFT in the reference is just this, exactly).
  - KEY reassociation: out = gather(conv) @ out_w = gather(conv @ out_w).
    A = conv @ ow is computed once at bin granularity (512 rows instead of
    4096), then the gather IS the final matmul: out(t,e) = S.T @ A.
  - out-projection partial per core over its 512 channels; host sums the
    two head-group partials per batch and adds out_b.
"""

import os
import sys
from contextlib import ExitStack

import numpy as np

for _p in ("/opt/trn_rl_repo", "/root/.axon_site/_ro/trn_rl_repo"):
    if os.path.isdir(_p) and _p not in sys.path:
        sys.path.append(_p)

import concourse.bacc as bacc
import concourse.mybir as mybir
import concourse.tile as tile
from concourse.bass_utils import run_bass_kernel_spmd

B, N, D = 4, 4096, 1024
H, HD, G = 16, 64, 512
SIGMA = 3.0
P = 128
KT = D // P          # 8 contraction tiles over D
TT = N // P          # 32 token tiles
GT = G // P          # 4 bin tiles
CLOC = 512           # channels per core (8 heads)
HLOC = CLOC // HD    # 8 heads per core
ECH = D // 512       # 2 chunks of out-channels for 512-wide psum
NCORES = 8

F32 = mybir.dt.float32
F32R = mybir.dt.float32r

# set by test harness to capture a profile; kernel() stores results here
TRACE = False
LAST_RESULT = None


def _field_idx():
    # exactly mirrors the reference (fp32 div then mul, trunc, clip)
    pos = np.arange(N, dtype=np.float32) / np.float32(N - 1) * np.float32(G - 1)
    return np.clip(pos.astype(np.int32), 0, G - 1)


def _causal_kernel():
    i = np.arange(G)
    dist = np.abs(i - G // 2)
    ker = np.where(i >= G // 2, 0.0, np.exp(-dist / SIGMA)).astype(np.float32)
    ker = ker / (ker.sum() + 1e-8)
    return ker


def _plans():
    idx = _field_idx()
    ker = _causal_kernel()
    gg = (np.arange(G)[None, :] - np.arange(G)[:, None]) % G  # CT[g, g2] = ker[(g2-g)%G]
    CTm = ker[gg].astype(np.float32)

    Smat = np.zeros((N, G), np.float32)
    Smat[np.arange(N), idx] = 1.0
    STm = np.ascontiguousarray(Smat.T)

    tt_gts = [sorted(set((idx[t * P:(t + 1) * P] // P).tolist())) for t in range(TT)]
    contribs = {gt: [t for t in range(TT) if gt in tt_gts[t]] for gt in range(GT)}
    scatter_plan = [
        [(gt, t == contribs[gt][0], t == contribs[gt][-1]) for gt in tt_gts[t]]
        for t in range(TT)
    ]
    return idx, CTm, Smat, STm, scatter_plan, tt_gts


def _build_program(with_kb, with_vb, scatter_plan, tt_gts):
    nc = bacc.Bacc("TRN2", target_bir_lowering=False, debug=False,
                   num_devices=NCORES)
    xT = nc.dram_tensor("xT", [D, N], F32R, kind="ExternalInput").ap()
    kwT = nc.dram_tensor("kwT", [D, CLOC], F32R, kind="ExternalInput").ap()
    vwT = nc.dram_tensor("vwT", [D, CLOC], F32R, kind="ExternalInput").ap()
    owT = nc.dram_tensor("owT", [CLOC, D], F32R, kind="ExternalInput").ap()
    Sm = nc.dram_tensor("Smat", [N, G], F32R, kind="ExternalInput").ap()
    STmat = nc.dram_tensor("STm", [G, N], F32R, kind="ExternalInput").ap()
    CTmat = nc.dram_tensor("CTm", [G, G], F32R, kind="ExternalInput").ap()
    kb = nc.dram_tensor("kb", [1, CLOC], F32R, kind="ExternalInput").ap() if with_kb else None
    vb = nc.dram_tensor("vb", [1, CLOC], F32R, kind="ExternalInput").ap() if with_vb else None
    out_d = nc.dram_tensor("out", [N, D], F32, kind="ExternalOutput").ap()

    xT_r = xT.rearrange("(kt p) n -> p kt n", p=P)
    kwT_r = kwT.rearrange("(kt p) c -> p kt c", p=P)
    vwT_r = vwT.rearrange("(kt p) c -> p kt c", p=P)

    with tile.TileContext(nc) as tc, ExitStack() as es:
        cpool = es.enter_context(tc.tile_pool(name="const", bufs=1))

        # resident tensors; k/v weights split per-kt so the first projection
        # matmuls only wait on their own 256KB slice (subtile deps).
        # Queue order matters: the HWDGE queues drain in issue order, so the
        # first token tile's x block goes out first, then weights round-robin
        # over the three DMA-capable queues; ow/ct are deferred to mid-loop.
        kw_sb = cpool.tile([P, KT, CLOC], F32R)
        vw_sb = cpool.tile([P, KT, CLOC], F32R)
        ow_sb = cpool.tile([P, GT, D], F32R)
        ct_sb = cpool.tile([P, GT, G], F32R)
        field_sb = cpool.tile([P, GT, G], F32R)
        convT_sb = cpool.tile([P, GT, G], F32R)
        A_sb = cpool.tile([P, GT, D], F32R)
        if with_kb or with_vb:
            ones_sb = cpool.tile([1, P], F32R)
            nc.vector.memset(ones_sb[:], 1.0)
        if with_kb:
            kb_sb = cpool.tile([1, CLOC], F32R)
            nc.sync.dma_start(kb_sb[:], kb[:])
        if with_vb:
            vb_sb = cpool.tile([1, CLOC], F32R)
            nc.sync.dma_start(vb_sb[:], vb[:])

        # ---- phase 1: projections, ||k||, wv, scatter ----
        ph1 = ExitStack()
        xpool = ph1.enter_context(tc.tile_pool(name="xin", bufs=3))
        spool = ph1.enter_context(tc.tile_pool(name="sblk", bufs=3))
        wvpool = ph1.enter_context(tc.tile_pool(name="wv", bufs=3))
        smpool = ph1.enter_context(tc.tile_pool(name="small", bufs=3))
        ps_k = ph1.enter_context(tc.tile_pool(name="ps_k", bufs=2, space="PSUM"))
        ps_v = ph1.enter_context(tc.tile_pool(name="ps_v", bufs=2, space="PSUM"))
        ps_f = ph1.enter_context(tc.tile_pool(name="ps_f", bufs=1, space="PSUM"))

        field_ps = [ps_f.tile([P, CLOC], F32, tag=f"fld{gt}", name=f"fld{gt}")
                    for gt in range(GT)]

        def emit_scatter(tt, wv):
            tsl = slice(tt * P, (tt + 1) * P)
            for gt, first, last in scatter_plan[tt]:
                sblk = spool.tile([P, P], F32R, tag="sblk")
                nc.gpsimd.dma_start(sblk[:], Sm[tsl, gt * P:(gt + 1) * P])
                nc.tensor.matmul(field_ps[gt][:], sblk[:], wv[:],
                                 start=first, stop=last)

        pending = None
        engs = (nc.sync, nc.scalar, nc.gpsimd)
        for tt in range(TT):
            tsl = slice(tt * P, (tt + 1) * P)
            xb = xpool.tile([P, KT, P], F32R, tag="xblk")
            if tt == 0:
                # split the first load so matmul kt=0 can start after 64KB
                for kt in range(KT):
                    engs[kt % 3].dma_start(xb[:, kt, :], xT_r[:, kt, tsl])
                # weights stream in behind it, round-robin over all queues
                for kt in range(KT):
                    engs[(2 * kt) % 3].dma_start(kw_sb[:, kt, :], kwT_r[:, kt, :])
                    engs[(2 * kt + 1) % 3].dma_start(vw_sb[:, kt, :], vwT_r[:, kt, :])
            else:
                nc.sync.dma_start(xb[:], xT_r[:, :, tsl])
            if tt == 8:
                # phase-2/3 constants, needed much later
                nc.gpsimd.dma_start(ow_sb[:], owT.rearrange("(ct p) e -> p ct e", p=P))
                nc.gpsimd.dma_start(ct_sb[:], CTmat.rearrange("(gt p) g2 -> p gt g2", p=P))

            kps = ps_k.tile([P, CLOC], F32, tag="kps")
            vps = ps_v.tile([P, CLOC], F32, tag="vps")
            for kt in range(KT):
                lhs = xb[:, kt, :]
                nc.tensor.matmul(kps[:], lhs, kw_sb[:, kt, :],
                                 start=(kt == 0), stop=(kt == KT - 1 and not with_kb))
                nc.tensor.matmul(vps[:], lhs, vw_sb[:, kt, :],
                                 start=(kt == 0), stop=(kt == KT - 1 and not with_vb))
            if with_kb:
                nc.tensor.matmul(kps[:], ones_sb[:], kb_sb[:], start=False, stop=True)
            if with_vb:
                nc.tensor.matmul(vps[:], ones_sb[:], vb_sb[:], start=False, stop=True)

            # scatter for the previous tile (keeps PE dense: its wv is ready)
            if pending is not None:
                emit_scatter(*pending)

            # ||k|| per head
            ksq = smpool.tile([P, CLOC], F32, tag="ksq")
            nc.scalar.activation(ksq[:], kps[:], mybir.ActivationFunctionType.Square)
            km2 = smpool.tile([P, HLOC], F32, tag="km2")
            nc.vector.reduce_sum(km2[:], ksq[:].rearrange("p (h d) -> p h d", d=HD),
                                 axis=mybir.AxisListType.X)
            km = smpool.tile([P, HLOC], F32, tag="km")
            nc.scalar.sqrt(km[:], km2[:])

            # wv = v * ||k||, one DVE op via stride-0 broadcast of km
            wv = wvpool.tile([P, CLOC], F32R, tag="wv")
            nc.vector.tensor_tensor(
                wv[:].rearrange("p (h d) -> p h d", d=HD),
                vps[:].rearrange("p (h d) -> p h d", d=HD),
                km[:].unsqueeze(2).broadcast_to((P, HLOC, HD)),
                mybir.AluOpType.mult)
            pending = (tt, wv)

        emit_scatter(*pending)
        for gt in range(GT):
            if gt % 2 == 0:
                nc.vector.tensor_copy(field_sb[:, gt, :], field_ps[gt][:])
            else:
                nc.scalar.copy(field_sb[:, gt, :], field_ps[gt][:])
        ph1.close()

        # ---- phase 2: convT = field.T @ C.T, then A = conv @ ow ----
        ph2 = ExitStack()
        ps_2 = ph2.enter_context(tc.tile_pool(name="ps_2", bufs=2, space="PSUM"))
        for ct in range(GT):
            cps = ps_2.tile([P, G], F32, tag="cps", bufs=2)
            for gt in range(GT):
                nc.tensor.matmul(cps[:], field_sb[:, gt, ct * P:(ct + 1) * P],
                                 ct_sb[:, gt, :],
                                 start=(gt == 0), stop=(gt == GT - 1))
            if ct % 2 == 0:
                nc.vector.tensor_copy(convT_sb[:, ct, :], cps[:])
            else:
                nc.scalar.copy(convT_sb[:, ct, :], cps[:])
        for gp in range(GT):
            aps = ps_2.tile([P, D], F32, tag="aps", bufs=2)
            for ec in range(ECH):
                esl = slice(ec * 512, (ec + 1) * 512)
                for ct in range(GT):
                    nc.tensor.matmul(aps[:, esl], convT_sb[:, ct, gp * P:(gp + 1) * P],
                                     ow_sb[:, ct, esl],
                                     start=(ct == 0), stop=(ct == GT - 1))
            if gp % 2 == 0:
                nc.vector.tensor_copy(A_sb[:, gp, :], aps[:])
            else:
                nc.scalar.copy(A_sb[:, gp, :], aps[:])
        ph2.close()

        # ---- phase 3: out(t, e) = S.T @ A  (gather == final matmul) ----
        ph3 = ExitStack()
        stpool = ph3.enter_context(tc.tile_pool(name="st_in", bufs=8))
        opool = ph3.enter_context(tc.tile_pool(name="osb", bufs=3))
        ps_o = ph3.enter_context(tc.tile_pool(name="ps_o", bufs=3, space="PSUM"))

        # prefetch every S.T block up front so the gather matmuls never wait
        st_tiles = []
        for tt in range(TT):
            tsl = slice(tt * P, (tt + 1) * P)
            per_tt = {}
            for gt in tt_gts[tt]:
                st = stpool.tile([P, P], F32R, tag=f"st{tt % 8}", name=f"st_{tt}_{gt}")
                nc.gpsimd.dma_start(st[:], STmat[gt * P:(gt + 1) * P, tsl])
                per_tt[gt] = st
            st_tiles.append(per_tt)

        for tt in range(TT):
            tsl = slice(tt * P, (tt + 1) * P)
            gts = tt_gts[tt]
            osb = opool.tile([P, D], F32, tag="osb")
            ops = ps_o.tile([P, D], F32, tag="ops")
            for ec in range(ECH):
                esl = slice(ec * 512, (ec + 1) * 512)
                for i, gt in enumerate(gts):
                    nc.tensor.matmul(ops[:, esl], st_tiles[tt][gt][:], A_sb[:, gt, esl],
                                     start=(i == 0), stop=(i == len(gts) - 1))
            if tt % 3 == 0:
                nc.scalar.copy(osb[:], ops[:])
            else:
                nc.vector.tensor_copy(osb[:], ops[:])
            nc.sync.dma_start(out_d[tsl, :], osb[:])
        ph3.close()

    nc.compile()
    return nc


_PROGRAM_CACHE = {}


def _get_program(with_kb, with_vb):
    key = (with_kb, with_vb)
    if key not in _PROGRAM_CACHE:
        _, _, _, _, sp, tg = _plans()
        _PROGRAM_CACHE[key] = _build_program(with_kb, with_vb, sp, tg)
    return _PROGRAM_CACHE[key]


def kernel(x, q_w, q_b, k_w, k_b, v_w, v_b, out_w, out_b):
    global LAST_RESULT
    x = np.asarray(x, dtype=np.float32)
    k_w = np.asarray(k_w, dtype=np.float32)
    k_b = np.asarray(k_b, dtype=np.float32)
    v_w = np.asarray(v_w, dtype=np.float32)
    v_b = np.asarray(v_b, dtype=np.float32)
    out_w = np.asarray(out_w, dtype=np.float32)
    out_b = np.asarray(out_b, dtype=np.float32)

    with_kb = bool(np.any(k_b))
    with_vb = bool(np.any(v_b))
    nc = _get_program(with_kb, with_vb)
    _, CTm, Smat, STm, _, _ = _plans()

    in_maps = []
    for c in range(NCORES):
        b, hg = c // 2, c % 2
        chs = slice(hg * CLOC, (hg + 1) * CLOC)
        m = {
            "xT": np.ascontiguousarray(x[b].T),
            "kwT": np.ascontiguousarray(k_w[chs, :].T),
            "vwT": np.ascontiguousarray(v_w[chs, :].T),
            "owT": np.ascontiguousarray(out_w[:, chs].T),
            "Smat": Smat,
            "STm": STm,
            "CTm": CTm,
        }
        if with_kb:
            m["kb"] = np.ascontiguousarray(k_b[chs][None, :])
        if with_vb:
            m["vb"] = np.ascontiguousarray(v_b[chs][None, :])
        in_maps.append(m)

    res = run_bass_kernel_spmd(nc, in_maps, core_ids=list(range(NCORES)),
                               trace=TRACE)
    LAST_RESULT = res

    out = np.empty((B, N, D), dtype=np.float32)
    for b in range(B):
        out[b] = res.results[2 * b]["out"] + res.results[2 * b + 1]["out"]
        out[b] += out_b[None, :]
    return out
